# revision 48
# baseline (speedup 1.0000x reference)
"""Trainium2 Bass kernel for nn_MultiHeadDilatedState.

Sharding: data-parallel over batch (B=8 -> 8 cores, one sequence per core).
Weights replicated. Per-core dataflow is channel-major [768, 4096], fp16
activations with fp32 PSUM accumulation; the two sigmoid-damped matmuls
(GLU gate half, mix gate) run in fp8e4m3 with DoubleRow (2x K per
instruction); sigmoid-scale compensation undoes the fp8 range scaling.

  x is pre-transposed and cast (fp16 + scaled fp8) on the host; x tiles
  stream per 512-token s-tile (no PE transposes on device).
  Phase A: router (fp16) + GLU: gate fp8-DR, value fp16, ACT sigmoid,
  DVE mul -> fp16 hbuf. Head weights stay in SBUF (no DRAM roundtrip).
  Phase B: 3 conv stages as diagonal-tap matmuls, in-place over fp16
  hbuf (descending s-tiles keep the update causal); the residual+tap0
  are one diagonal tap with weight (1+w0), so the PSUM evacuation is a
  single ACT/DVE copy(+bias) and stages alternate the two engines.
  Stage 2 interleaves the head-weight multiply (erep replication
  matmul + DVE mul) so the B->C boundary never idles the PE.
  Phase C/D software-pipelined per s-tile: fp8-DR mix-gate matmul +
  sigmoid -> o16; final matmul with the activation stationary so the
  output leaves token-major; mixing_b is added on the host.
"""

import os
import numpy as np

import concourse.bass as bass
import concourse.bacc as bacc
import concourse.mybir as mybir
import concourse.tile as tile
from concourse.bass_utils import run_bass_kernel_spmd

B, S, HID = 8, 4096, 768
NH, HD, KT = 12, 64, 4  # heads, head_dim, kernel taps
NC = 6                  # 768 / 128 channel chunks
ST = 512                # token tile
NST = S // ST           # 8
F32 = mybir.dt.float32
F16 = mybir.dt.float16
F8 = mybir.dt.float8e4
DR = mybir.MatmulPerfMode.DoubleRow
SIG = mybir.ActivationFunctionType.Sigmoid
IDENT = mybir.ActivationFunctionType.Identity
MGW_SCALE = 2048.0   # host scale on fp8 mix-gate weights
H8_SCALE = 4.0       # runtime scale on fp8 h activations
GW8_SCALE = 2048.0   # host scale on fp8 GLU-gate / router weights
X8_SCALE = 32.0      # host scale on fp8 x activations
X8W8_SCALE = GW8_SCALE * X8_SCALE

DILATIONS = [(1, 2, 4), (1, 1, 1), (4, 8, 16), (8, 16, 32), (32, 64, 128),
             (64, 128, 256), (256, 512, 1024), (1, 100, 200), (1, 500, 1000),
             (1, 1024, 2048), (3, 9, 27), (5, 25, 125)]


def build_bass():
    nc = bacc.Bacc()

    xt_d = nc.dram_tensor("xt16", [128, NC, S], F16, kind="ExternalInput")
    xt8_d = nc.dram_tensor("xt8", [128, NC // 2, 2, S], F8, kind="ExternalInput")
    gwT_d = nc.dram_tensor("gwTv", [128, NC, HID], F16, kind="ExternalInput")
    gw8_d = nc.dram_tensor("gw8", [128, NC // 2, 2, HID], F8, kind="ExternalInput")
    rwr_d = nc.dram_tensor("rwr", [128, NC, 64], F16, kind="ExternalInput")
    rb_d = nc.dram_tensor("rb", [NH, 1], F32, kind="ExternalInput")
    convdiag_d = nc.dram_tensor("convdiag", [128, 18, 256], F16, kind="ExternalInput")
    convbias_d = nc.dram_tensor("convbias", [128, NC, 3], F32, kind="ExternalInput")
    erep_d = nc.dram_tensor("erep", [NH, NC, 128], F16, kind="ExternalInput")
    mgb_d = nc.dram_tensor("mgb", [128, NC], F32, kind="ExternalInput")
    mgw_d = nc.dram_tensor("mgw8", [128, NC // 2, 2, HID], F8, kind="ExternalInput")
    mixt_d = nc.dram_tensor("mixt16", [128, NC, HID], F16, kind="ExternalInput")
    out_d = nc.dram_tensor("out", [S, HID], F32, kind="ExternalOutput")
    dbg_d = nc.dram_tensor("dbg", [NC, 128, S], F16, kind="ExternalOutput") if os.environ.get("KDBG") else None

    with tile.TileContext(nc) as tc:
        _body(tc, xt_d, xt8_d, gwT_d, gw8_d, rwr_d, rb_d, convdiag_d,
              convbias_d, erep_d, mgb_d, mgw_d, mixt_d, out_d, dbg_d)
    nc.finalize()
    return nc


def _body(tc, xt_d, xt8_d, gwT_d, gw8_d, rwr_d, rb_d, convdiag_d,
          convbias_d, erep_d, mgb_d, mgw_d, mixt_d, out_d, dbg_d=None):
    nc = tc.nc

    with (
        tc.tile_pool(name="persist", bufs=1) as persist,
        tc.tile_pool(name="sig", bufs=6) as p_sig,
        tc.tile_pool(name="o16p", bufs=3) as p_o16,
        tc.tile_pool(name="outsb", bufs=3) as p_out,
        tc.tile_pool(name="xt16p", bufs=4) as p_xt16,
        tc.tile_pool(name="xt8p", bufs=4) as p_xt8,
    ):
        # ---- persistent tiles ----
        # (128B-aligned tiles first: fp16 matmul stationary operands at SBUF
        # addresses not 0 mod 128 load corrupted weights.)
        gwT = persist.tile([128, NC, HID], F16, tag="gwT")
        gw8 = persist.tile([128, NC // 2, 2, HID], F8, tag="gw8")
        cvd = persist.tile([128, 18, 256], F16, tag="cvd16")
        mgw = persist.tile([128, NC // 2, 2, HID], F8, tag="mgw")
        mixt = persist.tile([128, NC, HID], F16, tag="mixt16")
        erep = persist.tile([NH, NC, 128], F16, tag="erep")
        rwr_p = persist.tile([128, NC, 64], F16, tag="rwr")
        rwr = rwr_p[:, :, 0:NH]
        hbuf = [persist.tile([128, S], F16, tag=f"h{c}", name=f"h{c}")
                for c in range(NC)]
        h8 = persist.tile([128, NC // 2, 2, S], F8, tag="h8")
        hws = persist.tile([NH, S], F16, tag="hws")
        convbias_p = persist.tile([128, NC, 16], F32, tag="convbias")
        convbias = convbias_p[:, :, 0:3]
        rb_p = persist.tile([NH, 32], F32, tag="rb")
        rb = rb_p[:, 0:1]
        mgb_p = persist.tile([128, 32], F32, tag="mgb")
        mgb = mgb_p[:, 0:NC]

        # ---- streamed x tiles (fp16 value path + fp8 gate/router path) ----
        xtiles = {}

        def issue_xt(st):
            s0 = st * ST
            t16 = p_xt16.tile([128, NC, ST], F16, tag="xt16")
            nc.sync.dma_start(t16[:, 0:3, :], xt_d[:, 0:3, s0:s0 + ST])
            nc.sync.dma_start(t16[:, 3:NC, :], xt_d[:, 3:NC, s0:s0 + ST])
            t8 = p_xt8.tile([128, NC // 2, 2, ST], F8, tag="xt8")
            nc.sync.dma_start(t8, xt8_d[:, :, :, s0:s0 + ST])
            xtiles[st] = (t16, t8)

        # DMA order: first x tiles + per-chunk gwT first so the PE can start
        # at ~10us and the GLU k-loop never starves.
        nc.sync.dma_start(rwr_p, rwr_d[:, :, :])
        nc.sync.dma_start(rb, rb_d[:, :])
        issue_xt(0)
        nc.sync.dma_start(gw8, gw8_d[:, :, :, :])
        for kc in range(NC):
            nc.sync.dma_start(gwT[:, kc, :], gwT_d[:, kc, :])
        issue_xt(1)
        nc.sync.dma_start(cvd, convdiag_d[:, :, :])
        nc.sync.dma_start(convbias, convbias_d[:, :, :])
        issue_xt(2)
        nc.sync.dma_start(erep, erep_d[:, :, :])
        nc.sync.dma_start(mgb, mgb_d[:, :])
        nc.sync.dma_start(mgw, mgw_d[:, :, :, :])
        nc.sync.dma_start(mixt, mixt_d[:, :, :])

        # ---- phase A: router + GLU (gate+router fp8 DoubleRow) ----
        with tc.tile_pool(name="psA", bufs=1, space="PSUM") as psA:
            for st in range(NST):
                s0 = st * ST
                if st + 3 < NST:
                    issue_xt(st + 3)
                t16, t8 = xtiles.pop(st)
                # router -> sigmoid -> head weights, kept in SBUF.
                # (fp16, not fp8: head-weight error multiplies the output
                # directly, so fp8 here doubles the final error.)
                pr = psA.tile([NH, ST], F32, tag="rtr", bufs=2)
                for kc in range(NC):
                    nc.tensor.matmul(pr[:, :], rwr[:, kc, :], t16[:, kc, :],
                                     start=(kc == 0), stop=(kc == NC - 1))
                nc.scalar.activation(hws[:, s0:s0 + ST], pr[:, :], SIG,
                                     bias=rb[:, :], scale=1.0)
                # GLU
                for oc in range(NC):
                    pg = psA.tile([128, ST], F32, tag="glu", bufs=4)
                    for kp in range(NC // 2):
                        nc.tensor.matmul(
                            pg[:, :], gw8[:, kp, :, oc * 128:(oc + 1) * 128],
                            t8[:, kp, :, :],
                            start=(kp == 0), stop=(kp == NC // 2 - 1),
                            perf_mode=DR)
                    sg = p_sig.tile([128, ST], F16, tag="sig")
                    nc.scalar.activation(sg[:, :], pg[:, :], SIG,
                                         scale=1.0 / X8W8_SCALE)
                    pv = psA.tile([128, ST], F32, tag="glu", bufs=4)
                    for kc in range(NC):
                        nc.tensor.matmul(
                            pv[:, :], gwT[:, kc, oc * 128:(oc + 1) * 128],
                            t16[:, kc, :],
                            start=(kc == 0), stop=(kc == NC - 1))
                    nc.vector.tensor_mul(hbuf[oc][:, s0:s0 + ST], pv[:, :], sg[:, :])

        if dbg_d is not None and os.environ.get("KDBG") == "A":
            for c in range(NC):
                nc.sync.dma_start(dbg_d[c, :, :], hbuf[c][:, :])

        # ---- phase B: 3 conv stages, in-place over fp16 hbuf ----
        # Tap 0 (shift 0) carries (1 + w0) so the residual is inside the
        # matmul; evacuation is one ACT copy(+bias). Descending s-tiles keep
        # the in-place update causal: taps m>=1 read strictly older tiles.
        with tc.tile_pool(name="psB", bufs=1, space="PSUM") as psB:
            for j in range(int(os.environ.get('KSTAGES', '3'))):
                for c in range(NC):
                    jc = j * NC + c
                    for st in reversed(range(NST)):
                        s0 = st * ST
                        pc = psB.tile([128, ST], F32, tag="conv",
                                      name=f"cv{j}_{c}_{st}", bufs=5)
                        mms = []
                        for half in (0, 1):
                            p0 = 64 * half
                            d = DILATIONS[2 * c + half][j]
                            first = True
                            for m in range(KT):
                                off = m * d
                                if off >= s0 + ST:
                                    continue
                                a = max(0, off - s0)
                                mms.append((p0, m, a, s0 - off + a, first))
                                first = False
                        # interleave the two quadrants so each LDWEIGHTS can
                        # be pulled ahead over the other quadrant's MM
                        ev = [x for x in mms if x[0] == 0]
                        od = [x for x in mms if x[0] == 64]
                        mms = []
                        for i in range(max(len(ev), len(od))):
                            if i < len(ev):
                                mms.append(ev[i])
                            if i < len(od):
                                mms.append(od[i])
                        nlast = {0: None, 64: None}
                        for i, (p0, m, a, r0, fi) in enumerate(mms):
                            nlast[p0] = i
                        for i, (p0, m, a, r0, fi) in enumerate(mms):
                            nc.tensor.matmul(
                                pc[p0:p0 + 64, a:ST],
                                cvd[p0:p0 + 64, jc, m * 64:(m + 1) * 64],
                                hbuf[c][p0:p0 + 64, r0:r0 + ST - a],
                                start=fi, stop=(i == nlast[p0]),
                                tile_position=(p0, p0))
                        # evacuation: alternate ACT/DVE in stages 0-1 so
                        # neither engine paces the PE; ACT-only in stage 2
                        # (DVE is busy with the interleaved B2 work there).
                        if j < 2 and c % 2 == 1:
                            nc.vector.tensor_scalar(
                                hbuf[c][:, s0:s0 + ST], pc[:, :],
                                1.0, convbias[:, c, j:j + 1],
                                mybir.AluOpType.mult, mybir.AluOpType.add)
                        else:
                            nc.scalar.activation(hbuf[c][:, s0:s0 + ST], pc[:, :],
                                                 IDENT, bias=convbias[:, c, j:j + 1],
                                                 scale=1.0)
                        if j == 2:
                            # B2 interleaved: multiply by head weights.
                            # Safe in-place: conv tiles < st never read
                            # column range st. The fp8 copy for the mix-gate
                            # matmul is staged here only for the first tile
                            # phase C consumes (st=7); the rest are
                            # prefetched inside the C loop to keep the DVE
                            # off this phase's critical path.
                            ph = psB.tile([128, ST], F32, tag="hwr", bufs=3)
                            nc.tensor.matmul(ph[:, :], erep[:, c, :],
                                             hws[:, s0:s0 + ST],
                                             start=True, stop=True)
                            nc.vector.tensor_mul(hbuf[c][:, s0:s0 + ST],
                                                 hbuf[c][:, s0:s0 + ST], ph[:, :])
                            if st == NST - 1:
                                nc.vector.tensor_scalar_mul(
                                    h8[:, c // 2, c % 2, s0:s0 + ST],
                                    hbuf[c][:, s0:s0 + ST], H8_SCALE)

        if dbg_d is not None and os.environ.get("KDBG") == "B":
            for c in range(NC):
                nc.sync.dma_start(dbg_d[c, :, :], hbuf[c][:, :])

        # ---- phases C + D per s-tile (B2 already ran inside conv j=2) ----
        # Descending st so the first tile's inputs (conv j=2 runs descending)
        # are ready early and the B->C boundary doesn't stall the PE.
        # The final output goes PSUM -> DRAM directly; mixing_b is added on
        # the host after gather.
        with tc.tile_pool(name="psC", bufs=1, space="PSUM") as psC:
            o16s = {}

            def phase_c(st):
                s0 = st * ST
                # mix gate (fp8 DoubleRow) -> fp16 o16 tiles
                o16 = p_o16.tile([128, NC, ST], F16, tag="o16")
                o16s[st] = o16
                for oc in range(NC):
                    pm = psC.tile([128, ST], F32, tag="mg", bufs=4)
                    for kp in range(NC // 2):
                        nc.tensor.matmul(
                            pm[:, :], mgw[:, kp, :, oc * 128:(oc + 1) * 128],
                            h8[:, kp, :, s0:s0 + ST],
                            start=(kp == 0), stop=(kp == NC // 2 - 1),
                            perf_mode=DR)
                    sg = p_sig.tile([128, ST], F16, tag="sig")
                    nc.scalar.activation(sg[:, :], pm[:, :], SIG,
                                         bias=mgb[:, oc:oc + 1],
                                         scale=1.0 / (MGW_SCALE * H8_SCALE))
                    nc.vector.tensor_mul(o16[:, oc, :],
                                         hbuf[oc][:, s0:s0 + ST], sg[:, :])
                # prefetch the next tile's fp8 h copy after this tile's DVE
                # muls (it is consumed a full tile later)
                if st > 0:
                    p0_ = (st - 1) * ST
                    for c in range(NC):
                        nc.vector.tensor_scalar_mul(
                            h8[:, c // 2, c % 2, p0_:p0_ + ST],
                            hbuf[c][:, p0_:p0_ + ST], H8_SCALE)

            def phase_d(st):
                # final matmul, activation stationary -> token-major out
                s0 = st * ST
                o16 = o16s.pop(st)
                for tl in range(4):
                    c0 = s0 + tl * 128
                    pmx = psC.tile([128, HID], F32, tag="mx", bufs=2)
                    for kc in range(NC):
                        nc.tensor.matmul(pmx[:, 0:512],
                                         o16[:, kc, tl * 128:(tl + 1) * 128],
                                         mixt[:, kc, 0:512],
                                         start=(kc == 0), stop=(kc == NC - 1))
                    for kc in range(NC):
                        nc.tensor.matmul(pmx[:, 512:HID],
                                         o16[:, kc, tl * 128:(tl + 1) * 128],
                                         mixt[:, kc, 512:HID],
                                         start=(kc == 0), stop=(kc == NC - 1))
                    osb = p_out.tile([128, HID], F32, tag="osb")
                    nc.scalar.copy(osb[:, :], pmx[:, :])
                    nc.sync.dma_start(out_d[c0:c0 + 128, :], osb[:, :])

            # software-pipelined by one tile: D(st) issues after C(st-1), so
            # the final matmuls fill the PE while C(st-1)'s sigmoid/mul
            # chain completes.
            phase_c(NST - 1)
            for st in reversed(range(NST - 1)):
                phase_c(st)
                phase_d(st + 1)
            phase_d(0)


def _q8(a, scale):
    import ml_dtypes
    return np.clip(np.asarray(a, dtype=np.float32) * scale,
                   -240, 240).astype(ml_dtypes.float8_e4m3)


def _prep_weights(gate_w, conv_w, conv_b, router_w, router_b,
                  mix_gate_w, mix_gate_b, mixing_w, mixing_b):
    f = np.float32
    h = np.float16
    # value half of the GLU in fp16: [128, kc, HID]
    gwT = np.ascontiguousarray(
        gate_w[0:HID].T.reshape(NC, 128, HID).transpose(1, 0, 2), dtype=h)
    # gate half in fp8 DoubleRow layout: [128, k-pair, 2, HID]
    gw8 = np.ascontiguousarray(
        _q8(gate_w[HID:].T, GW8_SCALE)
        .reshape(NC // 2, 2, 128, HID).transpose(2, 0, 1, 3))
    rwr = np.zeros((128, NC, 64), dtype=h)  # padded rows for DMA efficiency
    rwr[:, :, 0:NH] = router_w.T.reshape(NC, 128, NH).transpose(1, 0, 2)
    rb = np.ascontiguousarray(router_b.reshape(NH, 1), dtype=f)

    # fp16 tap diagonals: [128, 18, 256], (j,c) pair jc, tap m at cols m*64.
    # Tap m multiplies h shifted back by m*d and uses conv weight K-1-m;
    # tap 0 additionally carries the +1 residual.
    cd = np.zeros((128, 18, 256), dtype=h)
    ar = np.arange(HD)
    for j in range(3):
        for c in range(NC):
            for half in (0, 1):
                hd_ = 2 * c + half
                for m in range(KT):
                    w = conv_w[hd_, j, :, KT - 1 - m].astype(np.float32)
                    if m == 0:
                        w = w + 1.0
                    cd[half * HD + ar, j * NC + c, m * HD + ar] = w.astype(h)
    convdiag = np.ascontiguousarray(cd)
    cb = np.zeros((NC, 128, 3), dtype=f)
    for c in range(NC):
        for half in (0, 1):
            cb[c, half * HD:(half + 1) * HD, :] = conv_b[2 * c + half].T
    convbias = np.ascontiguousarray(cb.transpose(1, 0, 2), dtype=f)

    er = np.zeros((NH, NC, 128), dtype=h)
    for c in range(NC):
        for m in range(128):
            er[2 * c + (m >= HD), c, m] = 1.0

    mgb = np.ascontiguousarray(mix_gate_b.reshape(NC, 128).T, dtype=f)
    # fp8 DoubleRow mix-gate weights: [128, k-pair, 2, HID], scaled so the
    # 0.02-magnitude weights sit in e4m3's normal range.
    mgw8 = np.ascontiguousarray(
        _q8(mix_gate_w.T, MGW_SCALE)
        .reshape(NC // 2, 2, 128, HID).transpose(2, 0, 1, 3))
    mixt16 = np.ascontiguousarray(
        mixing_w.T.astype(h).reshape(NC, 128, HID).transpose(1, 0, 2))

    return {"gwTv": gwT, "gw8": gw8, "rwr": rwr, "rb": rb,
            "convdiag": convdiag, "convbias": convbias,
            "erep": er, "mgb": mgb, "mgw8": mgw8,
            "mixt16": mixt16}


_CACHE = {}


def _run(inputs, trace=False, tmpdir=None):
    if "nc" not in _CACHE:
        _CACHE["nc"] = build_bass()
    nc = _CACHE["nc"]

    w = _prep_weights(
        np.asarray(inputs["gate_w"]), np.asarray(inputs["conv_w"]),
        np.asarray(inputs["conv_b"]), np.asarray(inputs["router_w"]),
        np.asarray(inputs["router_b"]), np.asarray(inputs["mix_gate_w"]),
        np.asarray(inputs["mix_gate_b"]), np.asarray(inputs["mixing_w"]),
        np.asarray(inputs["mixing_b"]))
    x = np.asarray(inputs["x"], dtype=np.float32)

    in_maps = []
    for b in range(B):
        xTc = x[b].T.reshape(NC, 128, S)
        xt16 = np.ascontiguousarray(xTc.transpose(1, 0, 2), dtype=np.float16)
        xt8 = np.ascontiguousarray(
            _q8(xTc, X8_SCALE).reshape(NC // 2, 2, 128, S).transpose(2, 0, 1, 3))
        in_maps.append(dict(w, xt16=xt16, xt8=xt8))
    res = run_bass_kernel_spmd(nc, in_maps, core_ids=list(range(B)),
                               trace=trace, tmpdir=tmpdir)
    out = np.stack([res.results[b]["out"] for b in range(B)], axis=0)
    out = out + np.asarray(inputs["mixing_b"], dtype=np.float32)
    return out, res


def kernel(**inputs):
    out, _ = _run(inputs, trace=False)
    return out


if __name__ == "__main__":
    nc = build_bass()
    print("built ok; instructions:", len(nc.inst_map))


# revision 49
# speedup vs baseline: 1.2180x; 1.2180x over previous
"""Trainium2 Bass kernel for nn_MultiHeadDilatedState.

Sharding: data-parallel over batch (B=8 -> 8 cores, one sequence per core).
Weights replicated. Per-core dataflow is channel-major [768, 4096], fp16
activations with fp32 PSUM accumulation; the two sigmoid-damped matmuls
(GLU gate half, mix gate) run in fp8e4m3 with DoubleRow (2x K per
instruction); sigmoid-scale compensation undoes the fp8 range scaling.

  x is pre-transposed and cast (fp16 + scaled fp8) on the host; x tiles
  stream per 512-token s-tile (no PE transposes on device).
  Phase A: router (fp16) + GLU: gate fp8-DR, value fp16, ACT sigmoid,
  DVE mul -> fp16 hbuf. Head weights stay in SBUF (no DRAM roundtrip).
  Phase B: 3 conv stages as diagonal-tap matmuls, in-place over fp16
  hbuf (descending s-tiles keep the update causal); the residual+tap0
  are one diagonal tap with weight (1+w0), so the PSUM evacuation is a
  single ACT/DVE copy(+bias) and stages alternate the two engines.
  Stage 2 interleaves the head-weight multiply (erep replication
  matmul + DVE mul) so the B->C boundary never idles the PE.
  Phase C/D software-pipelined per s-tile: fp8-DR mix-gate matmul +
  sigmoid -> o16; final matmul with the activation stationary so the
  output leaves token-major; mixing_b is added on the host.
"""

import os
import numpy as np

import concourse.bass as bass
import concourse.bacc as bacc
import concourse.mybir as mybir
import concourse.tile as tile
from concourse.bass_utils import run_bass_kernel_spmd

B, S, HID = 8, 4096, 768
NH, HD, KT = 12, 64, 4  # heads, head_dim, kernel taps
NC = 6                  # 768 / 128 channel chunks
ST = 512                # token tile
NST = S // ST           # 8
F32 = mybir.dt.float32
F16 = mybir.dt.float16
F8 = mybir.dt.float8e4
DR = mybir.MatmulPerfMode.DoubleRow
SIG = mybir.ActivationFunctionType.Sigmoid
IDENT = mybir.ActivationFunctionType.Identity
MGW_SCALE = 2048.0   # host scale on fp8 mix-gate weights
H8_SCALE = 4.0       # runtime scale on fp8 h activations
GW8_SCALE = 2048.0   # host scale on fp8 GLU-gate / router weights
X8_SCALE = 32.0      # host scale on fp8 x activations
X8W8_SCALE = GW8_SCALE * X8_SCALE

DILATIONS = [(1, 2, 4), (1, 1, 1), (4, 8, 16), (8, 16, 32), (32, 64, 128),
             (64, 128, 256), (256, 512, 1024), (1, 100, 200), (1, 500, 1000),
             (1, 1024, 2048), (3, 9, 27), (5, 25, 125)]


def build_bass():
    nc = bacc.Bacc()

    xt_d = nc.dram_tensor("xt16", [128, NC, S], F16, kind="ExternalInput")
    xt8_d = nc.dram_tensor("xt8", [128, NC // 2, 2, S], F8, kind="ExternalInput")
    gwT_d = nc.dram_tensor("gwTv", [128, NC, HID], F16, kind="ExternalInput")
    gw8_d = nc.dram_tensor("gw8", [128, NC // 2, 2, HID], F8, kind="ExternalInput")
    rwr_d = nc.dram_tensor("rwr", [128, NC, 64], F16, kind="ExternalInput")
    rb_d = nc.dram_tensor("rb", [NH, 1], F32, kind="ExternalInput")
    convdiag_d = nc.dram_tensor("convdiag", [128, 18, 256], F16, kind="ExternalInput")
    convbias_d = nc.dram_tensor("convbias", [128, NC, 3], F32, kind="ExternalInput")
    erep_d = nc.dram_tensor("erep", [NH, NC, 128], F16, kind="ExternalInput")
    mgb_d = nc.dram_tensor("mgb", [128, NC], F32, kind="ExternalInput")
    mgw_d = nc.dram_tensor("mgw8", [128, NC // 2, 2, HID], F8, kind="ExternalInput")
    mixt_d = nc.dram_tensor("mixt16", [128, NC, HID], F16, kind="ExternalInput")
    out_d = nc.dram_tensor("out", [S, HID], F32, kind="ExternalOutput")
    dbg_d = nc.dram_tensor("dbg", [NC, 128, S], F16, kind="ExternalOutput") if os.environ.get("KDBG") else None

    with tile.TileContext(nc) as tc:
        _body(tc, xt_d, xt8_d, gwT_d, gw8_d, rwr_d, rb_d, convdiag_d,
              convbias_d, erep_d, mgb_d, mgw_d, mixt_d, out_d, dbg_d)
    nc.finalize()
    return nc


def _body(tc, xt_d, xt8_d, gwT_d, gw8_d, rwr_d, rb_d, convdiag_d,
          convbias_d, erep_d, mgb_d, mgw_d, mixt_d, out_d, dbg_d=None):
    nc = tc.nc

    with (
        tc.tile_pool(name="persist", bufs=1) as persist,
        tc.tile_pool(name="sig", bufs=6) as p_sig,
        tc.tile_pool(name="o16p", bufs=3) as p_o16,
        tc.tile_pool(name="outsb", bufs=3) as p_out,
        tc.tile_pool(name="xt16p", bufs=4) as p_xt16,
        tc.tile_pool(name="xt8p", bufs=4) as p_xt8,
    ):
        # ---- persistent tiles ----
        # (128B-aligned tiles first: fp16 matmul stationary operands at SBUF
        # addresses not 0 mod 128 load corrupted weights.)
        gwT = persist.tile([128, NC, HID], F16, tag="gwT")
        gw8 = persist.tile([128, NC // 2, 2, HID], F8, tag="gw8")
        cvd = persist.tile([128, 18, 256], F16, tag="cvd16")
        mgw = persist.tile([128, NC // 2, 2, HID], F8, tag="mgw")
        mixt = persist.tile([128, NC, HID], F16, tag="mixt16")
        erep = persist.tile([NH, NC, 128], F16, tag="erep")
        rwr_p = persist.tile([128, NC, 64], F16, tag="rwr")
        rwr = rwr_p[:, :, 0:NH]
        hbuf = [persist.tile([128, S], F16, tag=f"h{c}", name=f"h{c}")
                for c in range(NC)]
        h8 = persist.tile([128, NC // 2, 2, S], F8, tag="h8")
        hws = persist.tile([NH, S], F16, tag="hws")
        convbias_p = persist.tile([128, NC, 16], F32, tag="convbias")
        convbias = convbias_p[:, :, 0:3]
        rb_p = persist.tile([NH, 32], F32, tag="rb")
        rb = rb_p[:, 0:1]
        mgb_p = persist.tile([128, 32], F32, tag="mgb")
        mgb = mgb_p[:, 0:NC]

        # ---- streamed x tiles (fp16 value path + fp8 gate/router path) ----
        xtiles = {}

        def issue_xt(st):
            s0 = st * ST
            t16 = p_xt16.tile([128, NC, ST], F16, tag="xt16")
            nc.sync.dma_start(t16[:, 0:3, :], xt_d[:, 0:3, s0:s0 + ST])
            nc.sync.dma_start(t16[:, 3:NC, :], xt_d[:, 3:NC, s0:s0 + ST])
            t8 = p_xt8.tile([128, NC // 2, 2, ST], F8, tag="xt8")
            nc.sync.dma_start(t8, xt8_d[:, :, :, s0:s0 + ST])
            xtiles[st] = (t16, t8)

        # DMA order: first x tiles + per-chunk gwT first so the PE can start
        # at ~10us and the GLU k-loop never starves.
        issue_xt(0)
        nc.sync.dma_start(rwr_p, rwr_d[:, :, :])
        nc.sync.dma_start(rb, rb_d[:, :])
        nc.sync.dma_start(gw8, gw8_d[:, :, :, :])
        for kc in range(NC):
            nc.sync.dma_start(gwT[:, kc, :], gwT_d[:, kc, :])
        issue_xt(1)
        nc.sync.dma_start(cvd, convdiag_d[:, :, :])
        nc.sync.dma_start(convbias, convbias_d[:, :, :])
        issue_xt(2)
        nc.sync.dma_start(erep, erep_d[:, :, :])
        nc.sync.dma_start(mgb, mgb_d[:, :])
        nc.sync.dma_start(mgw, mgw_d[:, :, :, :])
        nc.sync.dma_start(mixt, mixt_d[:, :, :])

        # ---- phase A: router + GLU (gate+router fp8 DoubleRow) ----
        with tc.tile_pool(name="psA", bufs=1, space="PSUM") as psA:
            for st in range(NST):
                s0 = st * ST
                if st + 3 < NST:
                    issue_xt(st + 3)
                t16, t8 = xtiles.pop(st)
                # router -> sigmoid -> head weights, kept in SBUF.
                # (fp16, not fp8: head-weight error multiplies the output
                # directly, so fp8 here doubles the final error.)
                pr = psA.tile([NH, ST], F32, tag="rtr", bufs=2)
                for kc in range(NC):
                    nc.tensor.matmul(pr[:, :], rwr[:, kc, :], t16[:, kc, :],
                                     start=(kc == 0), stop=(kc == NC - 1))
                nc.scalar.activation(hws[:, s0:s0 + ST], pr[:, :], SIG,
                                     bias=rb[:, :], scale=1.0)
                # GLU
                for oc in range(NC):
                    pg = psA.tile([128, ST], F32, tag="glu", bufs=4)
                    for kp in range(NC // 2):
                        nc.tensor.matmul(
                            pg[:, :], gw8[:, kp, :, oc * 128:(oc + 1) * 128],
                            t8[:, kp, :, :],
                            start=(kp == 0), stop=(kp == NC // 2 - 1),
                            perf_mode=DR)
                    sg = p_sig.tile([128, ST], F16, tag="sig")
                    nc.scalar.activation(sg[:, :], pg[:, :], SIG,
                                         scale=1.0 / X8W8_SCALE)
                    pv = psA.tile([128, ST], F32, tag="glu", bufs=4)
                    for kc in range(NC):
                        nc.tensor.matmul(
                            pv[:, :], gwT[:, kc, oc * 128:(oc + 1) * 128],
                            t16[:, kc, :],
                            start=(kc == 0), stop=(kc == NC - 1))
                    nc.vector.tensor_mul(hbuf[oc][:, s0:s0 + ST], pv[:, :], sg[:, :])

        if dbg_d is not None and os.environ.get("KDBG") == "A":
            for c in range(NC):
                nc.sync.dma_start(dbg_d[c, :, :], hbuf[c][:, :])

        # ---- phase B: 3 conv stages, in-place over fp16 hbuf ----
        # Tap 0 (shift 0) carries (1 + w0) so the residual is inside the
        # matmul; evacuation is one ACT copy(+bias). Descending s-tiles keep
        # the in-place update causal: taps m>=1 read strictly older tiles.
        with tc.tile_pool(name="psB", bufs=1, space="PSUM") as psB:
            for j in range(int(os.environ.get('KSTAGES', '3'))):
                for c in range(NC):
                    jc = j * NC + c
                    for st in reversed(range(NST)):
                        s0 = st * ST
                        pc = psB.tile([128, ST], F32, tag="conv",
                                      name=f"cv{j}_{c}_{st}", bufs=5)
                        mms = []
                        for half in (0, 1):
                            p0 = 64 * half
                            d = DILATIONS[2 * c + half][j]
                            first = True
                            for m in range(KT):
                                off = m * d
                                if off >= s0 + ST:
                                    continue
                                a = max(0, off - s0)
                                mms.append((p0, m, a, s0 - off + a, first))
                                first = False
                        # interleave the two quadrants so each LDWEIGHTS can
                        # be pulled ahead over the other quadrant's MM
                        ev = [x for x in mms if x[0] == 0]
                        od = [x for x in mms if x[0] == 64]
                        mms = []
                        for i in range(max(len(ev), len(od))):
                            if i < len(ev):
                                mms.append(ev[i])
                            if i < len(od):
                                mms.append(od[i])
                        nlast = {0: None, 64: None}
                        for i, (p0, m, a, r0, fi) in enumerate(mms):
                            nlast[p0] = i
                        for i, (p0, m, a, r0, fi) in enumerate(mms):
                            nc.tensor.matmul(
                                pc[p0:p0 + 64, a:ST],
                                cvd[p0:p0 + 64, jc, m * 64:(m + 1) * 64],
                                hbuf[c][p0:p0 + 64, r0:r0 + ST - a],
                                start=fi, stop=(i == nlast[p0]),
                                tile_position=(p0, p0))
                        # evacuation: alternate ACT/DVE in stages 0-1 so
                        # neither engine paces the PE; ACT-only in stage 2
                        # (DVE is busy with the interleaved B2 work there).
                        if j < 2 and c % 2 == 1:
                            nc.vector.tensor_scalar(
                                hbuf[c][:, s0:s0 + ST], pc[:, :],
                                1.0, convbias[:, c, j:j + 1],
                                mybir.AluOpType.mult, mybir.AluOpType.add)
                        else:
                            nc.scalar.activation(hbuf[c][:, s0:s0 + ST], pc[:, :],
                                                 IDENT, bias=convbias[:, c, j:j + 1],
                                                 scale=1.0)
                        if j == 2:
                            # B2 interleaved: multiply by head weights.
                            # Safe in-place: conv tiles < st never read
                            # column range st. The fp8 copy for the mix-gate
                            # matmul is staged here only for the first tile
                            # phase C consumes (st=7); the rest are
                            # prefetched inside the C loop to keep the DVE
                            # off this phase's critical path.
                            ph = psB.tile([128, ST], F32, tag="hwr", bufs=3)
                            nc.tensor.matmul(ph[:, :], erep[:, c, :],
                                             hws[:, s0:s0 + ST],
                                             start=True, stop=True)
                            nc.vector.tensor_mul(hbuf[c][:, s0:s0 + ST],
                                                 hbuf[c][:, s0:s0 + ST], ph[:, :])
                            if st == NST - 1:
                                nc.vector.tensor_scalar_mul(
                                    h8[:, c // 2, c % 2, s0:s0 + ST],
                                    hbuf[c][:, s0:s0 + ST], H8_SCALE)

        if dbg_d is not None and os.environ.get("KDBG") == "B":
            for c in range(NC):
                nc.sync.dma_start(dbg_d[c, :, :], hbuf[c][:, :])

        # ---- phases C + D per s-tile (B2 already ran inside conv j=2) ----
        # Descending st so the first tile's inputs (conv j=2 runs descending)
        # are ready early and the B->C boundary doesn't stall the PE.
        # The final output goes PSUM -> DRAM directly; mixing_b is added on
        # the host after gather.
        with tc.tile_pool(name="psC", bufs=1, space="PSUM") as psC:
            o16s = {}

            def phase_c(st):
                s0 = st * ST
                # mix gate (fp8 DoubleRow) -> fp16 o16 tiles
                o16 = p_o16.tile([128, NC, ST], F16, tag="o16")
                o16s[st] = o16
                for oc in range(NC):
                    pm = psC.tile([128, ST], F32, tag="mg", bufs=4)
                    for kp in range(NC // 2):
                        nc.tensor.matmul(
                            pm[:, :], mgw[:, kp, :, oc * 128:(oc + 1) * 128],
                            h8[:, kp, :, s0:s0 + ST],
                            start=(kp == 0), stop=(kp == NC // 2 - 1),
                            perf_mode=DR)
                    sg = p_sig.tile([128, ST], F16, tag="sig")
                    nc.scalar.activation(sg[:, :], pm[:, :], SIG,
                                         bias=mgb[:, oc:oc + 1],
                                         scale=1.0 / (MGW_SCALE * H8_SCALE))
                    nc.vector.tensor_mul(o16[:, oc, :],
                                         hbuf[oc][:, s0:s0 + ST], sg[:, :])
                # prefetch the next tile's fp8 h copy after this tile's DVE
                # muls (it is consumed a full tile later)
                if st > 0:
                    p0_ = (st - 1) * ST
                    for c in range(NC):
                        nc.vector.tensor_scalar_mul(
                            h8[:, c // 2, c % 2, p0_:p0_ + ST],
                            hbuf[c][:, p0_:p0_ + ST], H8_SCALE)

            def phase_d(st):
                # final matmul, activation stationary -> token-major out
                s0 = st * ST
                o16 = o16s.pop(st)
                for tl in range(4):
                    c0 = s0 + tl * 128
                    pmx = psC.tile([128, HID], F32, tag="mx", bufs=2)
                    for kc in range(NC):
                        nc.tensor.matmul(pmx[:, 0:512],
                                         o16[:, kc, tl * 128:(tl + 1) * 128],
                                         mixt[:, kc, 0:512],
                                         start=(kc == 0), stop=(kc == NC - 1))
                    for kc in range(NC):
                        nc.tensor.matmul(pmx[:, 512:HID],
                                         o16[:, kc, tl * 128:(tl + 1) * 128],
                                         mixt[:, kc, 512:HID],
                                         start=(kc == 0), stop=(kc == NC - 1))
                    osb = p_out.tile([128, HID], F32, tag="osb")
                    nc.scalar.copy(osb[:, :], pmx[:, :])
                    nc.sync.dma_start(out_d[c0:c0 + 128, :], osb[:, :])

            # software-pipelined by one tile: D(st) issues after C(st-1), so
            # the final matmuls fill the PE while C(st-1)'s sigmoid/mul
            # chain completes.
            phase_c(NST - 1)
            for st in reversed(range(NST - 1)):
                phase_c(st)
                phase_d(st + 1)
            phase_d(0)


def _q8(a, scale):
    import ml_dtypes
    return np.clip(np.asarray(a, dtype=np.float32) * scale,
                   -240, 240).astype(ml_dtypes.float8_e4m3)


def _prep_weights(gate_w, conv_w, conv_b, router_w, router_b,
                  mix_gate_w, mix_gate_b, mixing_w, mixing_b):
    f = np.float32
    h = np.float16
    # value half of the GLU in fp16: [128, kc, HID]
    gwT = np.ascontiguousarray(
        gate_w[0:HID].T.reshape(NC, 128, HID).transpose(1, 0, 2), dtype=h)
    # gate half in fp8 DoubleRow layout: [128, k-pair, 2, HID]
    gw8 = np.ascontiguousarray(
        _q8(gate_w[HID:].T, GW8_SCALE)
        .reshape(NC // 2, 2, 128, HID).transpose(2, 0, 1, 3))
    rwr = np.zeros((128, NC, 64), dtype=h)  # padded rows for DMA efficiency
    rwr[:, :, 0:NH] = router_w.T.reshape(NC, 128, NH).transpose(1, 0, 2)
    rb = np.ascontiguousarray(router_b.reshape(NH, 1), dtype=f)

    # fp16 tap diagonals: [128, 18, 256], (j,c) pair jc, tap m at cols m*64.
    # Tap m multiplies h shifted back by m*d and uses conv weight K-1-m;
    # tap 0 additionally carries the +1 residual.
    cd = np.zeros((128, 18, 256), dtype=h)
    ar = np.arange(HD)
    for j in range(3):
        for c in range(NC):
            for half in (0, 1):
                hd_ = 2 * c + half
                for m in range(KT):
                    w = conv_w[hd_, j, :, KT - 1 - m].astype(np.float32)
                    if m == 0:
                        w = w + 1.0
                    cd[half * HD + ar, j * NC + c, m * HD + ar] = w.astype(h)
    convdiag = np.ascontiguousarray(cd)
    cb = np.zeros((NC, 128, 3), dtype=f)
    for c in range(NC):
        for half in (0, 1):
            cb[c, half * HD:(half + 1) * HD, :] = conv_b[2 * c + half].T
    convbias = np.ascontiguousarray(cb.transpose(1, 0, 2), dtype=f)

    er = np.zeros((NH, NC, 128), dtype=h)
    for c in range(NC):
        for m in range(128):
            er[2 * c + (m >= HD), c, m] = 1.0

    mgb = np.ascontiguousarray(mix_gate_b.reshape(NC, 128).T, dtype=f)
    # fp8 DoubleRow mix-gate weights: [128, k-pair, 2, HID], scaled so the
    # 0.02-magnitude weights sit in e4m3's normal range.
    mgw8 = np.ascontiguousarray(
        _q8(mix_gate_w.T, MGW_SCALE)
        .reshape(NC // 2, 2, 128, HID).transpose(2, 0, 1, 3))
    mixt16 = np.ascontiguousarray(
        mixing_w.T.astype(h).reshape(NC, 128, HID).transpose(1, 0, 2))

    return {"gwTv": gwT, "gw8": gw8, "rwr": rwr, "rb": rb,
            "convdiag": convdiag, "convbias": convbias,
            "erep": er, "mgb": mgb, "mgw8": mgw8,
            "mixt16": mixt16}


_CACHE = {}


def _run(inputs, trace=False, tmpdir=None):
    if "nc" not in _CACHE:
        _CACHE["nc"] = build_bass()
    nc = _CACHE["nc"]

    w = _prep_weights(
        np.asarray(inputs["gate_w"]), np.asarray(inputs["conv_w"]),
        np.asarray(inputs["conv_b"]), np.asarray(inputs["router_w"]),
        np.asarray(inputs["router_b"]), np.asarray(inputs["mix_gate_w"]),
        np.asarray(inputs["mix_gate_b"]), np.asarray(inputs["mixing_w"]),
        np.asarray(inputs["mixing_b"]))
    x = np.asarray(inputs["x"], dtype=np.float32)

    in_maps = []
    for b in range(B):
        xTc = x[b].T.reshape(NC, 128, S)
        xt16 = np.ascontiguousarray(xTc.transpose(1, 0, 2), dtype=np.float16)
        xt8 = np.ascontiguousarray(
            _q8(xTc, X8_SCALE).reshape(NC // 2, 2, 128, S).transpose(2, 0, 1, 3))
        in_maps.append(dict(w, xt16=xt16, xt8=xt8))
    res = run_bass_kernel_spmd(nc, in_maps, core_ids=list(range(B)),
                               trace=trace, tmpdir=tmpdir)
    out = np.stack([res.results[b]["out"] for b in range(B)], axis=0)
    out = out + np.asarray(inputs["mixing_b"], dtype=np.float32)
    return out, res


def kernel(**inputs):
    out, _ = _run(inputs, trace=False)
    return out


if __name__ == "__main__":
    nc = build_bass()
    print("built ok; instructions:", len(nc.inst_map))
